# revision 18
# baseline (speedup 1.0000x reference)
"""Trainium2 Bass kernel for nn_mtl_net (MLP -> GRU -> MLP -> per-cultivar heads).

Strategy (8 NeuronCores, SPMD):
- Tokens use t-major order: global token = 32*t + b (t timestep, b sample).
- P0: L1+L2 computed transposed (out2.T), token-sharded (each core 16 timesteps),
  then AllGather of out2.T.
- P1: GRU input projection xp.T, sharded over gate rows (each core owns 768 of
  6144 gate rows: a 256-row slice of each of r/z/n), full token range.
- P2: the sequential GRU scan, weight-sharded: each core computes its 256
  h-dims for the full batch each step, then a per-step AllGather of h.T
  rebuilds the full state on every core. PyTorch gate order (r,z,n).
  L3 (out_s = relu(h @ W3.T), PEN-sharded) is folded into the scan: its
  matmuls run in the PE idle time while the AllGather is in flight.
- P3: AllToAll re-shards out_s.T from pen-sharded to sample-sharded, then each
  core computes the 7 expert heads (gathered by cultivar label on host) for
  its 4 samples.
Host does only layout prep: transposes, bf16 casts, label-gather of head
tables, and final unshard/reshape.
"""
import functools
import numpy as np

import concourse.bacc as bacc
import concourse.tile as tile
import concourse.mybir as mybir
from concourse.bass_utils import run_bass_kernel_spmd

R = 8          # cores
B = 32         # batch
T = 128        # timesteps
D_IN = 32
H1 = 1024
H = 2048
PEN = 1024
NG = 768       # per-core gate rows (256 each of r,z,n)
BL = B // R    # local samples for heads

BF16 = mybir.dt.bfloat16
F32 = mybir.dt.float32
npbf16 = mybir.dt.np(BF16)
AF = mybir.ActivationFunctionType


def build_nc(Tn=T):
    TOKn = B * (Tn // R)     # local tokens in P0
    NTOKn = B * Tn           # all tokens
    HTOK = BL * Tn           # head-phase local tokens (4 samples x Tn)
    nc = bacc.Bacc("TRN2", target_bir_lowering=False, debug=False, num_devices=R)

    # --- inputs (per-core; several differ per core) ---
    xT = nc.dram_tensor("xT", [D_IN, TOKn], BF16, kind="ExternalInput")
    w1t = nc.dram_tensor("w1t", [D_IN, H1], BF16, kind="ExternalInput")
    b1t = nc.dram_tensor("b1t", [128, H1 // 128], F32, kind="ExternalInput")
    w2t = nc.dram_tensor("w2t", [H1, H], BF16, kind="ExternalInput")
    b2t = nc.dram_tensor("b2t", [128, H // 128], F32, kind="ExternalInput")
    wiht = nc.dram_tensor("wiht", [H, NG], BF16, kind="ExternalInput")
    xpbias = nc.dram_tensor("xpbias", [128, 6], F32, kind="ExternalInput")
    whht = nc.dram_tensor("whht", [H, NG], BF16, kind="ExternalInput")
    bhhn = nc.dram_tensor("bhhn", [1, 256], BF16, kind="ExternalInput")
    onesv = nc.dram_tensor("onesv", [1, 32], BF16, kind="ExternalInput")
    hzero = nc.dram_tensor("hzero", [128, 512], BF16, kind="ExternalInput")
    w3tc = nc.dram_tensor("w3tc", [H, 128], BF16, kind="ExternalInput")
    b3c = nc.dram_tensor("b3c", [128, 1], F32, kind="ExternalInput")
    wcat = nc.dram_tensor("wcat", [BL * 8 * 128, 7], BF16, kind="ExternalInput")
    headb = nc.dram_tensor("headb", [7, BL], F32, kind="ExternalInput")

    # --- outputs (per-core: 4 samples' heads) ---
    out_lin = nc.dram_tensor("out_lin", [3, HTOK], F32, kind="ExternalOutput")
    out_phd = nc.dram_tensor("out_phd", [4, HTOK], F32, kind="ExternalOutput")

    # --- internal DRAM ---
    ag2_in = nc.dram_tensor("ag2_in", [H, TOKn], BF16)
    ag2_out = nc.dram_tensor("ag2_out", [H * R, TOKn], BF16, addr_space="Shared")
    xpTs = [nc.dram_tensor(f"xpT{c}", [NG, TOKn], BF16) for c in range(R)]
    agh_in = nc.dram_tensor("agh_in", [Tn, 128, 64], BF16)
    agh_out = nc.dram_tensor("agh_out", [Tn, 1024, 64], BF16, addr_space="Shared")
    a2a_in = nc.dram_tensor("a2a_in", [8 * 128, HTOK], BF16)
    a2a_out = nc.dram_tensor("a2a_out", [8 * 128, HTOK], BF16)

    grp = [list(range(R))]

    with tile.TileContext(nc) as tc:
        # ---------------- P0: L1 + L2 (token-sharded, transposed) ----------
        with (
            tc.tile_pool(name="p0", bufs=1) as p0,
            tc.tile_pool(name="p0ps", bufs=2, space="PSUM") as p0ps,
        ):
            xsb = p0.tile([D_IN, TOKn], BF16)
            nc.sync.dma_start(xsb[:], xT[:])
            w1sb = p0.tile([D_IN, H1], BF16)
            nc.sync.dma_start(w1sb[:], w1t[:])
            b1sb = p0.tile([128, H1 // 128], F32)
            nc.sync.dma_start(b1sb[:], b1t[:])
            w2sb = p0.tile([128, H1 // 128, H], BF16)
            nc.sync.dma_start(w2sb[:], w2t.ap().rearrange("(k p) m -> p k m", p=128))
            b2sb = p0.tile([128, H // 128], F32)
            nc.sync.dma_start(b2sb[:], b2t[:])

            out1 = p0.tile([128, H1 // 128, TOKn], BF16)
            for m in range(H1 // 128):
                ps = p0ps.tile([128, TOKn], F32, tag="ps0")
                nc.tensor.matmul(ps[:], w1sb[:, 128 * m : 128 * m + 128], xsb[:],
                                 start=True, stop=True)
                nc.scalar.activation(out1[:, m, :], ps[:], AF.Relu,
                                     bias=b1sb[:, m : m + 1])
            out2 = p0.tile([128, H // 128, TOKn], BF16)
            for m in range(H // 128):
                ps = p0ps.tile([128, TOKn], F32, tag="ps0")
                for k in range(H1 // 128):
                    nc.tensor.matmul(ps[:], w2sb[:, k, 128 * m : 128 * m + 128],
                                     out1[:, k, :],
                                     start=(k == 0), stop=(k == H1 // 128 - 1))
                nc.scalar.activation(out2[:, m, :], ps[:], AF.Relu,
                                     bias=b2sb[:, m : m + 1])
            nc.sync.dma_start(ag2_in.ap().rearrange("(k p) m -> p k m", p=128),
                              out2[:])
        nc.gpsimd.collective_compute(
            "AllGather", mybir.AluOpType.bypass, replica_groups=grp,
            ins=[ag2_in[:]], outs=[ag2_out[:]],
        )

        # ---------------- P2: GRU scan with folded L3 ----------------------
        with (
            tc.tile_pool(name="p2w", bufs=1) as p2w,
            tc.tile_pool(name="p2h", bufs=3) as p2h,
            tc.tile_pool(name="p2xp", bufs=4) as p2xp,
            tc.tile_pool(name="p2g", bufs=2) as p2g,
            tc.tile_pool(name="p1mov", bufs=2) as p1mov,
            tc.tile_pool(name="p2ps", bufs=2, space="PSUM") as p2ps,
            tc.tile_pool(name="p2l3ps", bufs=1, space="PSUM") as p2l3ps,
            tc.tile_pool(name="p1ps", bufs=1, space="PSUM") as p1ps,
        ):
            wihsb = p2w.tile([128, H // 128, NG], BF16)
            nc.sync.dma_start(wihsb[:], wiht.ap().rearrange("(k p) m -> p k m", p=128))
            xpbsb = p2w.tile([128, 6], F32)
            nc.sync.dma_start(xpbsb[:], xpbias[:])

            def p1_mov_load(c8):
                mov = p1mov.tile([128, H // 128, TOKn], BF16, tag="mov",
                                 name=f"mov_{c8}")
                nc.scalar.dma_start(
                    mov[:],
                    ag2_out.ap()[H * c8 : H * (c8 + 1), :].rearrange(
                        "(k p) m -> p k m", p=128),
                )
                return mov

            p1_ps_live = {}

            def p1_unit(c8, m, mov, half=None):
                key = (c8, m)
                nk = H // 128
                if half is None or half == 0:
                    p1_ps_live[key] = p1ps.tile([128, TOKn], F32, tag="ps1",
                                                name=f"ps1_{c8}_{m}")
                ps = p1_ps_live[key]
                ks = range(nk) if half is None else (
                    range(nk // 2) if half == 0 else range(nk // 2, nk))
                for k in ks:
                    nc.tensor.matmul(ps[:], wihsb[:, k, 128 * m : 128 * m + 128],
                                     mov[:, k, :],
                                     start=(k == 0), stop=(k == nk - 1))
                if half is None or half == 1:
                    xo = p1mov.tile([128, TOKn], BF16, tag="xo", name=f"xo_{c8}_{m}")
                    nc.vector.tensor_scalar_add(xo[:], ps[:], xpbsb[:, m : m + 1])
                    nc.sync.dma_start(
                        xpTs[c8].ap()[128 * m : 128 * m + 128, :], xo[:])
            whhsb = p2w.tile([128, H // 128, NG], BF16)
            nc.sync.dma_start(whhsb[:], whht.ap().rearrange("(k p) m -> p k m", p=128))
            bhhnsb = p2w.tile([1, 256], BF16)
            nc.sync.dma_start(bhhnsb[:], bhhn[:])
            onesb = p2w.tile([1, 32], BF16)
            nc.sync.dma_start(onesb[:], onesv[:])
            w3sb = p2w.tile([128, H // 128, 128], BF16)
            nc.sync.dma_start(w3sb[:], w3tc.ap().rearrange("(k p) m -> p k m", p=128))
            b3sb = p2w.tile([128, 1], F32)
            nc.sync.dma_start(b3sb[:], b3c[:])
            # out_s.T local pen-chunk, b-major columns (col = Tn*?? no: 128*b + t)
            outsloc = p2w.tile([128, NTOKn], BF16)

            S = Tn // R  # timesteps per token chunk
            pre = 2 if S >= 8 else R  # chunks computed before the scan starts
            movs = {}
            for c8 in range(pre):
                movs[c8] = p1_mov_load(c8)
                for m in range(6):
                    p1_unit(c8, m, movs[c8])
            if pre < R:
                movs[pre] = p1_mov_load(pre)
            mov_at = {max(0, S * (c8 - 2) - 12): c8 for c8 in range(pre + 1, R)}
            units_at = {S * (c8 - 2) + 2 * m + 1 + h: (c8, m, h)
                        for c8 in range(pre, R) for m in range(6)
                        for h in range(2)}

            hTa = p2h.tile([128, 4, 64], BF16, tag="hTa")
            nc.sync.dma_start(hTa[:], hzero.ap()[:, 0:256].rearrange(
                "p (r m) -> p r m", r=4))
            hTb = p2h.tile([128, 4, 64], BF16, tag="hTb")
            nc.scalar.dma_start(hTb[:], hzero.ap()[:, 256:512].rearrange(
                "p (r m) -> p r m", r=4))
            hprev = p2g.tile([128, 64], BF16, tag="hnew")
            nc.vector.tensor_copy(hprev[:], hTa[:, 0, :])

            def rhs_chunk(k):
                half, kk = (hTa, k) if k < 8 else (hTb, k - 8)
                return half[:, kk // 2, 32 * (kk % 2) : 32 * (kk % 2) + 32]

            def l3_step(tau):
                # out_s.T[:, cols of timestep tau] from state tiles (h_tau)
                l3ps = p2l3ps.tile([128, 32], F32, tag="l3ps")
                for ki in range(H // 128):
                    nc.tensor.matmul(l3ps[:], w3sb[:, ki, :], rhs_chunk(ki),
                                     start=(ki == 0), stop=(ki == H // 128 - 1))
                nc.scalar.activation(outsloc[:, tau :: Tn], l3ps[:], AF.Relu,
                                     bias=b3sb[:, 0:1])

            for t in range(Tn):
                xps = p2xp.tile([128, 6, 32], BF16, tag="xps")
                tc8, tof = t // S, t % S
                nc.sync.dma_start(
                    xps[:],
                    xpTs[tc8].ap()[:, 32 * tof : 32 * tof + 32].rearrange(
                        "(m p) b -> p m b", p=128),
                )
                pss = []
                for g in range(3):
                    ps = p2ps.tile([128, 64], F32, tag=f"ps_{g}")
                    for j in range(2):
                        m = 2 * g + j
                        for k in range(H // 128):
                            nc.tensor.matmul(
                                ps[:, 32 * j : 32 * j + 32],
                                whhsb[:, k, 128 * m : 128 * m + 128],
                                rhs_chunk(k),
                                start=(k == 0),
                                stop=(k == H // 128 - 1 and g < 2),
                            )
                        if g == 2:  # add b_hh (n gate) via rank-1 matmul
                            nc.tensor.matmul(
                                ps[:, 32 * j : 32 * j + 32],
                                bhhnsb[0:1, 128 * j : 128 * j + 128],
                                onesb[0:1, :],
                                start=False, stop=True,
                            )
                    pss.append(ps)
                if t > 0:
                    l3_step(t - 1)  # h_{t-1} is the current state tiles
                ps_r, ps_z, ps_n = pss
                s_r = p2g.tile([128, 64], F32, tag="s_r")
                nc.vector.tensor_add(s_r[:], ps_r[:], xps[:, 0:2, :])
                r_g = p2g.tile([128, 64], F32, tag="r_g")
                nc.scalar.activation(r_g[:], s_r[:], AF.Sigmoid)
                s_z = p2g.tile([128, 64], F32, tag="s_z")
                nc.vector.tensor_add(s_z[:], ps_z[:], xps[:, 2:4, :])
                z_g = p2g.tile([128, 64], F32, tag="z_g")
                nc.scalar.activation(z_g[:], s_z[:], AF.Sigmoid)
                t1 = p2g.tile([128, 64], F32, tag="t1")
                nc.vector.tensor_mul(t1[:], r_g[:], ps_n[:])
                s_n = p2g.tile([128, 64], F32, tag="s_n")
                nc.vector.tensor_add(s_n[:], t1[:], xps[:, 4:6, :])
                n_g = p2g.tile([128, 64], F32, tag="n_g")
                nc.scalar.activation(n_g[:], s_n[:], AF.Tanh)
                d_g = p2g.tile([128, 64], F32, tag="d_g")
                nc.vector.tensor_sub(d_g[:], hprev[:], n_g[:])
                t2 = p2g.tile([128, 64], F32, tag="t2")
                nc.vector.tensor_mul(t2[:], z_g[:], d_g[:])
                hnew = p2g.tile([128, 64], BF16, tag="hnew")
                nc.vector.tensor_add(hnew[:], n_g[:], t2[:])

                nc.sync.dma_start(agh_in[t], hnew[:])
                nc.gpsimd.collective_compute(
                    "AllGather", mybir.AluOpType.bypass, replica_groups=grp,
                    ins=[agh_in[t]], outs=[agh_out[t]],
                )
                hprev = hnew
                if t in mov_at:
                    movs[mov_at[t]] = p1_mov_load(mov_at[t])
                if t in units_at:
                    uc, um, uh = units_at[t]
                    p1_unit(uc, um, movs[uc], half=uh)
                hTa = p2h.tile([128, 4, 64], BF16, tag="hTa")
                nc.sync.dma_start(hTa[:], agh_out[t][0:512, :].rearrange(
                    "(r p) m -> p r m", p=128))
                hTb = p2h.tile([128, 4, 64], BF16, tag="hTb")
                nc.scalar.dma_start(hTb[:], agh_out[t][512:1024, :].rearrange(
                    "(r p) m -> p r m", p=128))
            l3_step(Tn - 1)  # final state h_{Tn-1}

            # re-shard out_s.T: pen-sharded -> sample-sharded (AllToAll)
            nc.sync.dma_start(
                a2a_in.ap().rearrange("(j p) m -> p j m", p=128),
                outsloc[:].rearrange("p (j m) -> p j m", j=8),
            )
        nc.gpsimd.collective_compute(
            "AllToAll", mybir.AluOpType.bypass, replica_groups=grp,
            ins=[a2a_in[:]], outs=[a2a_out[:]],
        )

        # ---------------- P3: per-sample heads -----------------------------
        with (
            tc.tile_pool(name="p4", bufs=1) as p4,
            tc.tile_pool(name="p4ps", bufs=2, space="PSUM") as p4ps,
        ):
            outsT = p4.tile([128, 8, HTOK], BF16)
            nc.sync.dma_start(
                outsT[:], a2a_out.ap().rearrange("(r p) m -> p r m", p=128))
            wcatsb = p4.tile([128, BL, 8, 7], BF16)
            nc.sync.dma_start(
                wcatsb[:], wcat.ap().rearrange("(b k p) h -> p b k h", p=128, b=BL))
            headbsb = p4.tile([3, BL], F32)
            nc.sync.dma_start(headbsb[:], headb.ap()[0:3, :])
            headbsb_p = p4.tile([4, BL], F32)
            nc.sync.dma_start(headbsb_p[:], headb.ap()[3:7, :])
            lin_sb = p4.tile([3, HTOK], F32)
            ph_sb = p4.tile([4, HTOK], F32)
            for bl in range(BL):
                ps_l = p4ps.tile([3, Tn], F32, tag="ps_l")
                ps_p = p4ps.tile([4, Tn], F32, tag="ps_p")
                for ki in range(8):
                    mov_b = outsT[:, ki, Tn * bl : Tn * (bl + 1)]
                    nc.tensor.matmul(ps_l[:], wcatsb[:, bl, ki, 0:3], mov_b,
                                     start=(ki == 0), stop=(ki == 7))
                    nc.tensor.matmul(ps_p[:], wcatsb[:, bl, ki, 3:7], mov_b,
                                     start=(ki == 0), stop=(ki == 7))
                nc.scalar.activation(lin_sb[:, Tn * bl : Tn * (bl + 1)], ps_l[:],
                                     AF.Identity, bias=headbsb[:, bl : bl + 1])
                nc.scalar.activation(ph_sb[:, Tn * bl : Tn * (bl + 1)], ps_p[:],
                                     AF.Sigmoid, bias=headbsb_p[:, bl : bl + 1])
            nc.sync.dma_start(out_lin[:], lin_sb[:])
            nc.sync.dma_start(out_phd[:], ph_sb[:])

    nc.compile()
    return nc


@functools.lru_cache(maxsize=2)
def _built(Tn):
    return build_nc(Tn)


def make_inputs(x, cultivar_label, W1, b1, W2, b2, W_ih, b_ih, W_hh, b_hh,
                W3, b3, W4, b4, W5, b5, W6, b6, Wp, bp, Tn=T):
    TOKn = B * (Tn // R)
    f = lambda a: np.asarray(a, dtype=np.float32)
    x = f(x)[:, :Tn, :]
    labels = np.asarray(cultivar_label).reshape(-1).astype(np.int64)
    W1, b1, W2, b2 = f(W1), f(b1), f(W2), f(b2)
    W_ih, b_ih, W_hh, b_hh = f(W_ih), f(b_ih), f(W_hh), f(b_hh)
    W3, b3 = f(W3), f(b3)
    W4, b4, W5, b5, W6, b6 = f(W4), f(b4), f(W5), f(b5), f(W6), f(b6)
    Wp, bp = f(Wp), f(bp)

    bf = lambda a: np.ascontiguousarray(a).astype(npbf16)
    f32 = lambda a: np.ascontiguousarray(a).astype(np.float32)

    # x.T in t-major token order: col = 32*t + b
    xT_all = x.transpose(2, 1, 0).reshape(D_IN, Tn * B)  # [d, t*B+b]

    Wcat = np.concatenate(
        [W4[labels][:, None, :], W5[labels][:, None, :], W6[labels][:, None, :],
         Wp[:, labels].transpose(1, 0, 2)], axis=1)  # [B, 7, PEN]
    headb_full = np.concatenate(
        [np.stack([b4[labels], b5[labels], b6[labels]]), bp[:, labels]], axis=0)

    shared = {
        "w1t": bf(W1.T), "b1t": f32(b1.reshape(8, 128).T),
        "w2t": bf(W2.T), "b2t": f32(b2.reshape(16, 128).T),
        "onesv": bf(np.ones((1, 32))), "hzero": bf(np.zeros((128, 512))),
    }
    in_maps = []
    for c in range(R):
        idx = np.concatenate([np.arange(256 * c, 256 * c + 256) + g * H
                              for g in range(3)])
        fold = b_ih[idx].copy()
        fold[:512] += b_hh[idx][:512]
        wcat_c = Wcat[BL * c : BL * (c + 1)]  # [BL, 7, PEN]
        m = dict(shared)
        m["xT"] = bf(xT_all[:, TOKn * c : TOKn * (c + 1)])
        m["wiht"] = bf(W_ih[idx].T)
        m["xpbias"] = f32(fold.reshape(6, 128).T)
        m["whht"] = bf(W_hh[idx].T)
        m["bhhn"] = bf(b_hh[idx][512:].reshape(1, 256))
        m["w3tc"] = bf(W3.T[:, 128 * c : 128 * c + 128])
        m["b3c"] = f32(b3[128 * c : 128 * c + 128].reshape(128, 1))
        m["wcat"] = bf(wcat_c.transpose(0, 2, 1).reshape(BL, 8, 128, 7)
                       .reshape(-1, 7))
        m["headb"] = f32(headb_full[:, BL * c : BL * (c + 1)])
        in_maps.append(m)
    return in_maps


def run_device(in_maps, Tn=T):
    nc = _built(Tn)
    res = run_bass_kernel_spmd(nc, in_maps, list(range(R)))
    return res


def assemble(res, Tn=T):
    lin = np.stack([res.results[c]["out_lin"] for c in range(R)])  # [R,3,BL*Tn]
    ph = np.stack([res.results[c]["out_phd"] for c in range(R)])   # [R,4,BL*Tn]
    lin = lin.reshape(R, 3, BL, Tn).transpose(1, 0, 2, 3).reshape(3, B, Tn)
    ph = ph.reshape(R, 4, BL, Tn).transpose(1, 0, 2, 3).reshape(4, B, Tn)
    lt = lin[..., None].astype(np.float32)
    out_ph = ph[..., None].astype(np.float32)
    return (lt[0], lt[1], lt[2], out_ph, 0)


def kernel(**inputs):
    in_maps = make_inputs(**inputs)
    res = run_device(in_maps)
    return assemble(res)


# revision 19
# speedup vs baseline: 1.0271x; 1.0271x over previous
"""Trainium2 Bass kernel for nn_mtl_net (MLP -> GRU -> MLP -> per-cultivar heads).

Strategy (8 NeuronCores, SPMD):
- Tokens use t-major order: global token = 32*t + b (t timestep, b sample).
- P0: L1+L2 computed transposed (out2.T), token-sharded (each core 16 timesteps),
  then AllGather of out2.T.
- P1: GRU input projection xp.T, sharded over gate rows (each core owns 768 of
  6144 gate rows: a 256-row slice of each of r/z/n), full token range.
- P2: the sequential GRU scan, weight-sharded: each core computes its 256
  h-dims for the full batch each step, then a per-step AllGather of h.T
  rebuilds the full state on every core. PyTorch gate order (r,z,n).
  L3 (out_s = relu(h @ W3.T), PEN-sharded) is folded into the scan: its
  matmuls run in the PE idle time while the AllGather is in flight.
- P3: AllToAll re-shards out_s.T from pen-sharded to sample-sharded, then each
  core computes the 7 expert heads (gathered by cultivar label on host) for
  its 4 samples.
Host does only layout prep: transposes, bf16 casts, label-gather of head
tables, and final unshard/reshape.
"""
import functools
import numpy as np

import concourse.bacc as bacc
import concourse.tile as tile
import concourse.mybir as mybir
from concourse.bass_utils import run_bass_kernel_spmd

R = 8          # cores
B = 32         # batch
T = 128        # timesteps
D_IN = 32
H1 = 1024
H = 2048
PEN = 1024
NG = 768       # per-core gate rows (256 each of r,z,n)
BL = B // R    # local samples for heads

BF16 = mybir.dt.bfloat16
F32 = mybir.dt.float32
npbf16 = mybir.dt.np(BF16)
AF = mybir.ActivationFunctionType


def build_nc(Tn=T):
    TOKn = B * (Tn // R)     # local tokens in P0
    NTOKn = B * Tn           # all tokens
    HTOK = BL * Tn           # head-phase local tokens (4 samples x Tn)
    nc = bacc.Bacc("TRN2", target_bir_lowering=False, debug=False, num_devices=R)

    # --- inputs (per-core; several differ per core) ---
    xT = nc.dram_tensor("xT", [D_IN, TOKn], BF16, kind="ExternalInput")
    w1t = nc.dram_tensor("w1t", [D_IN, H1], BF16, kind="ExternalInput")
    b1t = nc.dram_tensor("b1t", [128, H1 // 128], F32, kind="ExternalInput")
    w2t = nc.dram_tensor("w2t", [H1, H], BF16, kind="ExternalInput")
    b2t = nc.dram_tensor("b2t", [128, H // 128], F32, kind="ExternalInput")
    wiht = nc.dram_tensor("wiht", [H, NG], BF16, kind="ExternalInput")
    xpbias = nc.dram_tensor("xpbias", [128, 6], F32, kind="ExternalInput")
    whht = nc.dram_tensor("whht", [H, NG], BF16, kind="ExternalInput")
    bhhn = nc.dram_tensor("bhhn", [1, 256], BF16, kind="ExternalInput")
    onesv = nc.dram_tensor("onesv", [1, 32], BF16, kind="ExternalInput")
    hzero = nc.dram_tensor("hzero", [128, 512], BF16, kind="ExternalInput")
    w3tc = nc.dram_tensor("w3tc", [H, 128], BF16, kind="ExternalInput")
    b3c = nc.dram_tensor("b3c", [128, 1], F32, kind="ExternalInput")
    wcat = nc.dram_tensor("wcat", [BL * 8 * 128, 7], BF16, kind="ExternalInput")
    headb = nc.dram_tensor("headb", [7, BL], F32, kind="ExternalInput")

    # --- outputs (per-core: 4 samples' heads) ---
    out_lin = nc.dram_tensor("out_lin", [3, HTOK], F32, kind="ExternalOutput")
    out_phd = nc.dram_tensor("out_phd", [4, HTOK], F32, kind="ExternalOutput")

    # --- internal DRAM ---
    ag2_in = nc.dram_tensor("ag2_in", [H, TOKn], BF16)
    ag2_out = nc.dram_tensor("ag2_out", [H * R, TOKn], BF16, addr_space="Shared")
    xpTs = [nc.dram_tensor(f"xpT{c}", [NG, TOKn], BF16) for c in range(R)]
    agh_in = nc.dram_tensor("agh_in", [Tn, 128, 64], BF16)
    agh_out = nc.dram_tensor("agh_out", [Tn, 1024, 64], BF16, addr_space="Shared")
    a2a_in = nc.dram_tensor("a2a_in", [8 * 128, HTOK], BF16)
    a2a_out = nc.dram_tensor("a2a_out", [8 * 128, HTOK], BF16)

    grp = [list(range(R))]

    with tile.TileContext(nc) as tc:
        # ---------------- P0: L1 + L2 (token-sharded, transposed) ----------
        with (
            tc.tile_pool(name="p0", bufs=1) as p0,
            tc.tile_pool(name="p0ps", bufs=2, space="PSUM") as p0ps,
        ):
            xsb = p0.tile([D_IN, TOKn], BF16)
            nc.sync.dma_start(xsb[:], xT[:])
            w1sb = p0.tile([D_IN, H1], BF16)
            nc.sync.dma_start(w1sb[:], w1t[:])
            b1sb = p0.tile([128, H1 // 128], F32)
            nc.sync.dma_start(b1sb[:], b1t[:])
            w2sb = p0.tile([128, H1 // 128, H], BF16)
            nc.sync.dma_start(w2sb[:], w2t.ap().rearrange("(k p) m -> p k m", p=128))
            b2sb = p0.tile([128, H // 128], F32)
            nc.sync.dma_start(b2sb[:], b2t[:])

            out1 = p0.tile([128, H1 // 128, TOKn], BF16)
            for m in range(H1 // 128):
                ps = p0ps.tile([128, TOKn], F32, tag="ps0")
                nc.tensor.matmul(ps[:], w1sb[:, 128 * m : 128 * m + 128], xsb[:],
                                 start=True, stop=True)
                nc.scalar.activation(out1[:, m, :], ps[:], AF.Relu,
                                     bias=b1sb[:, m : m + 1])
            out2 = p0.tile([128, H // 128, TOKn], BF16)
            for m in range(H // 128):
                ps = p0ps.tile([128, TOKn], F32, tag="ps0")
                for k in range(H1 // 128):
                    nc.tensor.matmul(ps[:], w2sb[:, k, 128 * m : 128 * m + 128],
                                     out1[:, k, :],
                                     start=(k == 0), stop=(k == H1 // 128 - 1))
                nc.scalar.activation(out2[:, m, :], ps[:], AF.Relu,
                                     bias=b2sb[:, m : m + 1])
            nc.sync.dma_start(ag2_in.ap().rearrange("(k p) m -> p k m", p=128),
                              out2[:])
        nc.gpsimd.collective_compute(
            "AllGather", mybir.AluOpType.bypass, replica_groups=grp,
            ins=[ag2_in[:]], outs=[ag2_out[:]],
        )

        # ---------------- P2: GRU scan with folded L3 ----------------------
        with (
            tc.tile_pool(name="p2w", bufs=1) as p2w,
            tc.tile_pool(name="p2h", bufs=3) as p2h,
            tc.tile_pool(name="p2xp", bufs=4) as p2xp,
            tc.tile_pool(name="p2g", bufs=2) as p2g,
            tc.tile_pool(name="p1mov", bufs=2) as p1mov,
            tc.tile_pool(name="p2ps", bufs=2, space="PSUM") as p2ps,
            tc.tile_pool(name="p2l3ps", bufs=1, space="PSUM") as p2l3ps,
            tc.tile_pool(name="p1ps", bufs=1, space="PSUM") as p1ps,
        ):
            wihsb = p2w.tile([128, H // 128, NG], BF16)
            nc.sync.dma_start(wihsb[:], wiht.ap().rearrange("(k p) m -> p k m", p=128))
            xpbsb = p2w.tile([128, 6], F32)
            nc.sync.dma_start(xpbsb[:], xpbias[:])

            def p1_mov_load(c8):
                mov = p1mov.tile([128, H // 128, TOKn], BF16, tag="mov",
                                 name=f"mov_{c8}")
                nc.scalar.dma_start(
                    mov[:],
                    ag2_out.ap()[H * c8 : H * (c8 + 1), :].rearrange(
                        "(k p) m -> p k m", p=128),
                )
                return mov

            p1_ps_live = {}

            def p1_unit(c8, m, mov, half=None):
                key = (c8, m)
                nk = H // 128
                if half is None or half == 0:
                    p1_ps_live[key] = p1ps.tile([128, TOKn], F32, tag="ps1",
                                                name=f"ps1_{c8}_{m}")
                ps = p1_ps_live[key]
                ks = range(nk) if half is None else (
                    range(nk // 2) if half == 0 else range(nk // 2, nk))
                for k in ks:
                    nc.tensor.matmul(ps[:], wihsb[:, k, 128 * m : 128 * m + 128],
                                     mov[:, k, :],
                                     start=(k == 0), stop=(k == nk - 1))
                if half is None or half == 1:
                    xo = p1mov.tile([128, TOKn], BF16, tag="xo", name=f"xo_{c8}_{m}")
                    nc.vector.tensor_scalar_add(xo[:], ps[:], xpbsb[:, m : m + 1])
                    nc.sync.dma_start(
                        xpTs[c8].ap()[128 * m : 128 * m + 128, :], xo[:])
            whhsb = p2w.tile([128, H // 128, NG], BF16)
            nc.sync.dma_start(whhsb[:], whht.ap().rearrange("(k p) m -> p k m", p=128))
            bhhnsb = p2w.tile([1, 256], BF16)
            nc.sync.dma_start(bhhnsb[:], bhhn[:])
            onesb = p2w.tile([1, 32], BF16)
            nc.sync.dma_start(onesb[:], onesv[:])
            w3sb = p2w.tile([128, H // 128, 128], BF16)
            nc.sync.dma_start(w3sb[:], w3tc.ap().rearrange("(k p) m -> p k m", p=128))
            b3sb = p2w.tile([128, 1], F32)
            nc.sync.dma_start(b3sb[:], b3c[:])
            # out_s.T local pen-chunk, b-major columns (col = Tn*?? no: 128*b + t)
            outsloc = p2w.tile([128, NTOKn], BF16)

            S = Tn // R  # timesteps per token chunk
            pre = 2 if S >= 8 else R  # chunks computed before the scan starts
            movs = {}
            for c8 in range(pre):
                movs[c8] = p1_mov_load(c8)
                for m in range(6):
                    p1_unit(c8, m, movs[c8])
            if pre < R:
                movs[pre] = p1_mov_load(pre)
            mov_at = {max(0, S * (c8 - 2) - 12): c8 for c8 in range(pre + 1, R)}
            units_at = {S * (c8 - 2) + 2 * m + 1 + h: (c8, m, h)
                        for c8 in range(pre, R) for m in range(6)
                        for h in range(2)}

            hTa = p2h.tile([128, 4, 64], BF16, tag="hTa")
            nc.sync.dma_start(hTa[:], hzero.ap()[:, 0:256].rearrange(
                "p (r m) -> p r m", r=4))
            hTb = p2h.tile([128, 4, 64], BF16, tag="hTb")
            nc.scalar.dma_start(hTb[:], hzero.ap()[:, 256:512].rearrange(
                "p (r m) -> p r m", r=4))
            hprev = p2g.tile([128, 64], BF16, tag="hnew")
            nc.vector.tensor_copy(hprev[:], hTa[:, 0, :])

            def rhs_chunk(k):
                half, kk = (hTa, k) if k < 8 else (hTb, k - 8)
                return half[:, kk // 2, 32 * (kk % 2) : 32 * (kk % 2) + 32]

            def l3_step(tau):
                # out_s.T[:, cols of timestep tau] from state tiles (h_tau)
                l3ps = p2l3ps.tile([128, 32], F32, tag="l3ps")
                for ki in range(H // 128):
                    nc.tensor.matmul(l3ps[:], w3sb[:, ki, :], rhs_chunk(ki),
                                     start=(ki == 0), stop=(ki == H // 128 - 1))
                nc.scalar.activation(outsloc[:, tau :: Tn], l3ps[:], AF.Relu,
                                     bias=b3sb[:, 0:1])

            for t in range(Tn):
                xps = p2xp.tile([128, 6, 32], BF16, tag="xps")
                tc8, tof = t // S, t % S
                nc.sync.dma_start(
                    xps[:],
                    xpTs[tc8].ap()[:, 32 * tof : 32 * tof + 32].rearrange(
                        "(m p) b -> p m b", p=128),
                )
                pss = []
                for g in range(3):
                    ps = p2ps.tile([128, 64], F32, tag=f"ps_{g}")
                    for j in range(2):
                        m = 2 * g + j
                        for k in range(H // 128):
                            nc.tensor.matmul(
                                ps[:, 32 * j : 32 * j + 32],
                                whhsb[:, k, 128 * m : 128 * m + 128],
                                rhs_chunk(k),
                                start=(k == 0),
                                stop=(k == H // 128 - 1 and g < 2),
                            )
                        if g == 2:  # add b_hh (n gate) via rank-1 matmul
                            nc.tensor.matmul(
                                ps[:, 32 * j : 32 * j + 32],
                                bhhnsb[0:1, 128 * j : 128 * j + 128],
                                onesb[0:1, :],
                                start=False, stop=True,
                            )
                    pss.append(ps)
                if t > 0:
                    l3_step(t - 1)  # h_{t-1} is the current state tiles
                ps_r, ps_z, ps_n = pss
                s_r = p2g.tile([128, 64], F32, tag="s_r")
                nc.vector.tensor_add(s_r[:], ps_r[:], xps[:, 0:2, :])
                r_g = p2g.tile([128, 64], F32, tag="r_g")
                nc.scalar.activation(r_g[:], s_r[:], AF.Sigmoid)
                s_z = p2g.tile([128, 64], F32, tag="s_z")
                nc.vector.tensor_add(s_z[:], ps_z[:], xps[:, 2:4, :])
                z_g = p2g.tile([128, 64], F32, tag="z_g")
                nc.scalar.activation(z_g[:], s_z[:], AF.Sigmoid)
                u_g = p2g.tile([128, 64], F32, tag="u_g")
                nc.vector.tensor_mul(u_g[:], z_g[:], hprev[:])
                v_g = p2g.tile([128, 64], F32, tag="v_g")
                nc.scalar.activation(v_g[:], s_z[:], AF.Sigmoid, scale=-1.0)
                t1 = p2g.tile([128, 64], F32, tag="t1")
                nc.vector.tensor_mul(t1[:], r_g[:], ps_n[:])
                s_n = p2g.tile([128, 64], F32, tag="s_n")
                nc.vector.tensor_add(s_n[:], t1[:], xps[:, 4:6, :])
                n_g = p2g.tile([128, 64], F32, tag="n_g")
                nc.scalar.activation(n_g[:], s_n[:], AF.Tanh)
                w_g = p2g.tile([128, 64], F32, tag="w_g")
                nc.vector.tensor_mul(w_g[:], v_g[:], n_g[:])
                hnew = p2g.tile([128, 64], BF16, tag="hnew")
                nc.vector.tensor_add(hnew[:], w_g[:], u_g[:])

                nc.sync.dma_start(agh_in[t], hnew[:])
                nc.gpsimd.collective_compute(
                    "AllGather", mybir.AluOpType.bypass, replica_groups=grp,
                    ins=[agh_in[t]], outs=[agh_out[t]],
                )
                hprev = hnew
                if t in mov_at:
                    movs[mov_at[t]] = p1_mov_load(mov_at[t])
                if t in units_at:
                    uc, um, uh = units_at[t]
                    p1_unit(uc, um, movs[uc], half=uh)
                hTa = p2h.tile([128, 4, 64], BF16, tag="hTa")
                nc.sync.dma_start(hTa[:], agh_out[t][0:512, :].rearrange(
                    "(r p) m -> p r m", p=128))
                hTb = p2h.tile([128, 4, 64], BF16, tag="hTb")
                nc.scalar.dma_start(hTb[:], agh_out[t][512:1024, :].rearrange(
                    "(r p) m -> p r m", p=128))
            l3_step(Tn - 1)  # final state h_{Tn-1}

            # re-shard out_s.T: pen-sharded -> sample-sharded (AllToAll)
            nc.sync.dma_start(
                a2a_in.ap().rearrange("(j p) m -> p j m", p=128),
                outsloc[:].rearrange("p (j m) -> p j m", j=8),
            )
        nc.gpsimd.collective_compute(
            "AllToAll", mybir.AluOpType.bypass, replica_groups=grp,
            ins=[a2a_in[:]], outs=[a2a_out[:]],
        )

        # ---------------- P3: per-sample heads -----------------------------
        with (
            tc.tile_pool(name="p4", bufs=1) as p4,
            tc.tile_pool(name="p4ps", bufs=2, space="PSUM") as p4ps,
        ):
            outsT = p4.tile([128, 8, HTOK], BF16)
            nc.sync.dma_start(
                outsT[:], a2a_out.ap().rearrange("(r p) m -> p r m", p=128))
            wcatsb = p4.tile([128, BL, 8, 7], BF16)
            nc.sync.dma_start(
                wcatsb[:], wcat.ap().rearrange("(b k p) h -> p b k h", p=128, b=BL))
            headbsb = p4.tile([3, BL], F32)
            nc.sync.dma_start(headbsb[:], headb.ap()[0:3, :])
            headbsb_p = p4.tile([4, BL], F32)
            nc.sync.dma_start(headbsb_p[:], headb.ap()[3:7, :])
            lin_sb = p4.tile([3, HTOK], F32)
            ph_sb = p4.tile([4, HTOK], F32)
            for bl in range(BL):
                ps_l = p4ps.tile([3, Tn], F32, tag="ps_l")
                ps_p = p4ps.tile([4, Tn], F32, tag="ps_p")
                for ki in range(8):
                    mov_b = outsT[:, ki, Tn * bl : Tn * (bl + 1)]
                    nc.tensor.matmul(ps_l[:], wcatsb[:, bl, ki, 0:3], mov_b,
                                     start=(ki == 0), stop=(ki == 7))
                    nc.tensor.matmul(ps_p[:], wcatsb[:, bl, ki, 3:7], mov_b,
                                     start=(ki == 0), stop=(ki == 7))
                nc.scalar.activation(lin_sb[:, Tn * bl : Tn * (bl + 1)], ps_l[:],
                                     AF.Identity, bias=headbsb[:, bl : bl + 1])
                nc.scalar.activation(ph_sb[:, Tn * bl : Tn * (bl + 1)], ps_p[:],
                                     AF.Sigmoid, bias=headbsb_p[:, bl : bl + 1])
            nc.sync.dma_start(out_lin[:], lin_sb[:])
            nc.sync.dma_start(out_phd[:], ph_sb[:])

    nc.compile()
    return nc


@functools.lru_cache(maxsize=2)
def _built(Tn):
    return build_nc(Tn)


def make_inputs(x, cultivar_label, W1, b1, W2, b2, W_ih, b_ih, W_hh, b_hh,
                W3, b3, W4, b4, W5, b5, W6, b6, Wp, bp, Tn=T):
    TOKn = B * (Tn // R)
    f = lambda a: np.asarray(a, dtype=np.float32)
    x = f(x)[:, :Tn, :]
    labels = np.asarray(cultivar_label).reshape(-1).astype(np.int64)
    W1, b1, W2, b2 = f(W1), f(b1), f(W2), f(b2)
    W_ih, b_ih, W_hh, b_hh = f(W_ih), f(b_ih), f(W_hh), f(b_hh)
    W3, b3 = f(W3), f(b3)
    W4, b4, W5, b5, W6, b6 = f(W4), f(b4), f(W5), f(b5), f(W6), f(b6)
    Wp, bp = f(Wp), f(bp)

    bf = lambda a: np.ascontiguousarray(a).astype(npbf16)
    f32 = lambda a: np.ascontiguousarray(a).astype(np.float32)

    # x.T in t-major token order: col = 32*t + b
    xT_all = x.transpose(2, 1, 0).reshape(D_IN, Tn * B)  # [d, t*B+b]

    Wcat = np.concatenate(
        [W4[labels][:, None, :], W5[labels][:, None, :], W6[labels][:, None, :],
         Wp[:, labels].transpose(1, 0, 2)], axis=1)  # [B, 7, PEN]
    headb_full = np.concatenate(
        [np.stack([b4[labels], b5[labels], b6[labels]]), bp[:, labels]], axis=0)

    shared = {
        "w1t": bf(W1.T), "b1t": f32(b1.reshape(8, 128).T),
        "w2t": bf(W2.T), "b2t": f32(b2.reshape(16, 128).T),
        "onesv": bf(np.ones((1, 32))), "hzero": bf(np.zeros((128, 512))),
    }
    in_maps = []
    for c in range(R):
        idx = np.concatenate([np.arange(256 * c, 256 * c + 256) + g * H
                              for g in range(3)])
        fold = b_ih[idx].copy()
        fold[:512] += b_hh[idx][:512]
        wcat_c = Wcat[BL * c : BL * (c + 1)]  # [BL, 7, PEN]
        m = dict(shared)
        m["xT"] = bf(xT_all[:, TOKn * c : TOKn * (c + 1)])
        m["wiht"] = bf(W_ih[idx].T)
        m["xpbias"] = f32(fold.reshape(6, 128).T)
        m["whht"] = bf(W_hh[idx].T)
        m["bhhn"] = bf(b_hh[idx][512:].reshape(1, 256))
        m["w3tc"] = bf(W3.T[:, 128 * c : 128 * c + 128])
        m["b3c"] = f32(b3[128 * c : 128 * c + 128].reshape(128, 1))
        m["wcat"] = bf(wcat_c.transpose(0, 2, 1).reshape(BL, 8, 128, 7)
                       .reshape(-1, 7))
        m["headb"] = f32(headb_full[:, BL * c : BL * (c + 1)])
        in_maps.append(m)
    return in_maps


def run_device(in_maps, Tn=T):
    nc = _built(Tn)
    res = run_bass_kernel_spmd(nc, in_maps, list(range(R)))
    return res


def assemble(res, Tn=T):
    lin = np.stack([res.results[c]["out_lin"] for c in range(R)])  # [R,3,BL*Tn]
    ph = np.stack([res.results[c]["out_phd"] for c in range(R)])   # [R,4,BL*Tn]
    lin = lin.reshape(R, 3, BL, Tn).transpose(1, 0, 2, 3).reshape(3, B, Tn)
    ph = ph.reshape(R, 4, BL, Tn).transpose(1, 0, 2, 3).reshape(4, B, Tn)
    lt = lin[..., None].astype(np.float32)
    out_ph = ph[..., None].astype(np.float32)
    return (lt[0], lt[1], lt[2], out_ph, 0)


def kernel(**inputs):
    in_maps = make_inputs(**inputs)
    res = run_device(in_maps)
    return assemble(res)
